# revision 22
# baseline (speedup 1.0000x reference)
"""Trainium2 Bass kernel for AggregatedInfluenceScorer — single fused launch.

Reference computation:
    a = actor_embeddings @ W_actor + b_actor            # [N=2048, D=256]
    b = bill_embeddings  @ W_bill  + b_bill             # [M=1024, D=256]
    scores[n,m] = sum_d w_score[d] * tanh(a[n,d] + b[m,d]) + b_score
    out[n] = mean_m(scores[n,m] * bill_outcomes[m])

tanh(a+b) on the box |a|,|b| <= 3 is approximated by a rank-12 separable
expansion  tanh(a+b) ~= sum_{j,k} C[j,k] F_j(a) F_k(b)  over the basis
F = {1, tanh(1.1x + t_1..7), relu(x + r_1..4)}  (C from a pinv fit on a 701-pt
grid; end-to-end rel err ~2.4e-3 incl. bf16/f32r device arithmetic, vs the
2e-2 gate).  The [N,M,D] intermediate collapses:

    out[n] = sum_j sum_d F_j(a[n,d]) h_j[d]  +  c0
    h_j[d] = (w[d]/M) * sum_k C[j,k] g_k[d]
    g_k[d] = sum_m outc[m] * F_k(b[m,d])

ONE SPMD launch on 8 cores (launch fixed cost ~13us, so the two-launch
host-reduced design pays it twice).  Per core: bills slice (128) -> partial
gT[d, k] -> remote_dma_broadcast of [128, 22] f32 to all 8 cores (XOR-relative
dests; block placement is sender-order-agnostic because the reduction sums all
blocks with an identical stationary) -> one matmul folds the 8-block sum, the
C mixing, 1/M, and the constant bill feature -> transposes + w-scale give the
contraction stationary -> actor slice (256) features (tanh on ScalarE, relu on
DVE/Pool in parallel) -> pair-packed contraction matmuls -> out slice [256].
The kernel-entry barrier (prelude AllGather) gates the first remote send so a
late-starting peer cannot lose the remote-sem increments to its init clear.

Tile's single-core scheduling sim cannot model the cross-core sem increments,
so the barrier wait and the remote-sem wait are emitted *after* scheduling and
spliced into the instruction stream right before their consumers.
"""

import os

import ml_dtypes
import numpy as np

import concourse.bass as bass
import concourse.bacc as bacc
import concourse.mybir as mybir
from concourse.tile import TileContext
from concourse.bass_utils import run_bass_kernel_spmd
from concourse import masks

F32 = mybir.dt.float32
F32R = mybir.dt.float32r
BF16 = mybir.dt.bfloat16
TANH = mybir.ActivationFunctionType.Tanh
ADD = mybir.AluOpType.add
MAX = mybir.AluOpType.max
MULT = mybir.AluOpType.mult

N_CORES = 8
N, M, D, E = 2048, 1024, 256, 512
NA, NB = N // N_CORES, M // N_CORES      # 256 actors, 128 bills per core
NT, NR = 7, 4                            # tanh units (ScalarE), relu units (DVE/Pool)
NFm1 = NT + NR                           # 11 non-constant features
T_SPAN, T_SCALE, R_SPAN = 3.1, 1.1, 2.8
BOX = 3.0
RCOND = 1e-7
MSW = 16                                 # misc width
PACKW = 40                               # packed stationary width per half


def _basis_params():
    k = np.arange(NT)
    t_sh = -T_SPAN * np.cos((k + 0.5) * np.pi / NT)          # tanh shifts
    r_off = -np.linspace(-R_SPAN, R_SPAN, NR)                # relu(x + r)
    return t_sh, r_off


def _feats_np(x):
    t_sh, r_off = _basis_params()
    x = np.asarray(x, np.float64)
    out = [np.ones_like(x)]
    out += [np.tanh(T_SCALE * x + t) for t in t_sh]
    out += [np.maximum(x + r, 0.0) for r in r_off]
    return np.stack(out, 0)


def _coeffs():
    g = np.linspace(-BOX, BOX, 701)
    Ga = _feats_np(g)                                        # [12, 701]
    F = np.tanh(g[:, None] + g[None, :])
    Gp = np.linalg.pinv(Ga.T, rcond=RCOND)
    return (Gp @ F @ Gp.T).astype(np.float64)                # [12, 12]


def _bf16(x):
    return np.asarray(x, np.float32).astype(ml_dtypes.bfloat16)


def _pack_ktiles(x, p=128):
    """[T*p, W] -> [p, T*W] with block t = x[t*p:(t+1)*p, :]."""
    T = x.shape[0] // p
    return np.ascontiguousarray(
        x.reshape(T, p, x.shape[1]).transpose(1, 0, 2).reshape(p, T * x.shape[1])
    )


def _build():
    nc = bacc.Bacc()
    BT_d = nc.dram_tensor("BT", [128, E], BF16, kind="ExternalInput")
    Wb_d = nc.dram_tensor("Wb", [128, 4 * D], BF16, kind="ExternalInput")
    AT_d = nc.dram_tensor("AT", [128, 2 * NA], BF16, kind="ExternalInput")
    Wa_d = nc.dram_tensor("Wa", [128, 2 * D], BF16, kind="ExternalInput")
    ms_d = nc.dram_tensor("misc", [128, MSW], F32, kind="ExternalInput")
    rvb_d = nc.dram_tensor("rvb", [1, 2 * D], BF16, kind="ExternalInput")
    wr_d = nc.dram_tensor("wrow", [1, D], F32, kind="ExternalInput")
    cts_d = nc.dram_tensor("cts", [128, 48], BF16, kind="ExternalInput")
    out_d = nc.dram_tensor("out", [1, NA], F32, kind="ExternalOutput")

    rs = nc.alloc_semaphore("rs_g")
    ls = nc.alloc_semaphore("ls_g")

    t_sh, r_off = _basis_params()

    with TileContext(nc) as tc:
        with (
            tc.tile_pool(name="cst", bufs=1) as cst,
            tc.tile_pool(name="psp", bufs=1, space=bass.MemorySpace.PSUM) as psp,
        ):
            # ---- input DMAs (bill-side first on each queue)
            bt = cst.tile([128, E], BF16)
            nc.sync.dma_start(bt[:], BT_d[:])
            wb = cst.tile([128, 4 * D], BF16)
            nc.scalar.dma_start(wb[:], Wb_d[:])
            ms = cst.tile([128, MSW], F32)
            nc.gpsimd.dma_start(ms[:], ms_d[:])
            at = cst.tile([128, 2 * NA], BF16)
            nc.sync.dma_start(at[:], AT_d[:])
            wa = cst.tile([128, 2 * D], BF16)
            nc.scalar.dma_start(wa[:], Wa_d[:])
            rvb = cst.tile([1, 2 * D], BF16)
            nc.gpsimd.dma_start(rvb[:], rvb_d[:])
            wrow = cst.tile([1, D], F32)
            nc.gpsimd.dma_start(wrow[:], wr_d[:])
            cts = cst.tile([128, 48], BF16)
            nc.gpsimd.dma_start(cts[:], cts_d[:])

            # ---- constants
            ones_bf = cst.tile([1, NA], BF16)
            nc.gpsimd.memset(ones_bf[:], 1.0)
            ident = cst.tile([128, 128], F32)
            masks.make_identity(nc, ident[:])
            junk = cst.tile([128, 128], BF16)
            nc.vector.memset(junk[:], 1.0)
            hT = cst.tile([128, 2 * PACKW], BF16)
            nc.vector.memset(hT[:], 0.0)
            tg8 = cst.tile([128, 2 * 128], BF16)
            nc.vector.memset(tg8[:], 0.0)
            # ones row (c-group a only) folds the constant bill feature
            nc.vector.memset(tg8[96:97, 0:128], 1.0)
            G8 = cst.tile([128, 8 * 2 * NFm1], F32)  # remote-written; never local

            # warm the ACT table + PE clock while DMAs run
            warm = cst.tile([1, 1], F32)
            nc.gpsimd.memset(warm[:], 0.0)
            nc.scalar.activation(warm[:], warm[:], TANH)
            wps = psp.tile([128, 128], F32, tag="wps")
            for _ in range(14):
                nc.tensor.matmul(wps[:], junk[:], junk[:], start=True, stop=True)

            # ---- bill projection ppb[m, d] (+bb)
            ppb = psp.tile([NB, D], F32, tag="ppb")
            for kk in range(4):
                nc.tensor.matmul(ppb[:], bt[:, kk * 128:(kk + 1) * 128],
                                 wb[:, kk * D:(kk + 1) * D],
                                 start=(kk == 0), stop=False)
            nc.tensor.matmul(ppb[:], ones_bf[:, 0:NB], rvb[:, D:2 * D],
                             start=False, stop=True)

            # ---- actor projection X[d_lo, h*256+n] (+ba)
            X = psp.tile([128, 2 * NA], F32, tag="X")
            for h in range(2):
                for kk in range(2):
                    nc.tensor.matmul(
                        X[:, h * NA:(h + 1) * NA],
                        wa[:, kk * D + h * 128:kk * D + (h + 1) * 128],
                        at[:, kk * NA:(kk + 1) * NA],
                        start=(kk == 0), stop=False)
                nc.tensor.matmul(X[:, h * NA:(h + 1) * NA],
                                 rvb[:, h * 128:(h + 1) * 128], ones_bf[:],
                                 start=False, stop=True)

            # ---- bill features Qt[m, jj*256 + h*128 + d_lo]  (f32r)
            Qt = cst.tile([NB, NFm1 * D], F32R)
            for jj in range(NT):           # tanh on ScalarE
                nc.scalar.activation(Qt[:, jj * D:(jj + 1) * D], ppb[:], TANH,
                                     bias=ms[:, jj:jj + 1], scale=T_SCALE)
            for i in range(NR):            # relu on DVE (Pool cannot read PSUM)
                jj = NT + i
                nc.vector.tensor_scalar(Qt[:, jj * D:(jj + 1) * D], ppb[:],
                                        ms[:, NT + i:NT + i + 1], 0.0, ADD, MAX)

            # ---- g matmuls: PGT[d_lo, 2*(h*11+jj)] = sum_m outc[m] Qt[m, ...]
            # (moving operand is outc duplicated to 2 columns — 1-wide moving
            # fails the ISA check — so each result lands twice; the GS copy
            # reads the even columns.)
            outc_r = cst.tile([NB, 2], F32R)
            nc.vector.tensor_copy(outc_r[:, 0:1], ms[:, 11:12])
            nc.vector.tensor_copy(outc_r[:, 1:2], ms[:, 11:12])
            PGT = psp.tile([128, 4 * NFm1], F32, tag="PGT")
            for jj in [NT, NT + 1, NT + 2, NT + 3] + list(range(NT)):
                for h in range(2):
                    col = 2 * (h * NFm1 + jj)
                    nc.tensor.matmul(
                        PGT[:, col:col + 2],
                        Qt[:, jj * D + h * 128:jj * D + h * 128 + 128],
                        outc_r[:], start=True, stop=True)
            GS = cst.tile([128, 2 * NFm1], F32)
            nc.vector.tensor_copy(
                GS[:], PGT[:].rearrange("p (c two) -> p two c", two=2)[:, 0, :])

            # ---- broadcast gT to all 8 cores (XOR-relative; slot = block)
            for dlt in range(8):
                rd = [(0, dlt) if k == dlt else None for k in range(8)]
                nc.gpsimd.remote_dma_broadcast(
                    G8[:, 22 * dlt:22 * dlt + 22], GS[:],
                    remote_sem=rs, local_sem=ls, rdests=rd)
            trig = nc.gpsimd.trigger_dma(count=None)

            # ---- actor features fv pairs [d_lo, h*512 + f*256 + n] (bf16)
            fvp = [cst.tile([128, 2 * 2 * NA], BF16, name=f"fvp{q}")
                   for q in range(5)]
            fvs = cst.tile([128, 2 * NA], BF16)
            Xv = X[:].rearrange("p (h n) -> p h n", h=2)
            acts = []
            for j in range(1, NFm1 + 1):
                q, f = divmod(j - 1, 2)
                if q < 5:
                    dst = fvp[q][:].rearrange("p (h f n) -> p h f n",
                                              h=2, f=2)[:, :, f, :]
                else:
                    dst = fvs[:].rearrange("p (h n) -> p h n", h=2)
                if j <= NT:
                    acts.append((nc.scalar, "act", j, dst))
                else:
                    acts.append((None, "relu", j, dst))
            for eng, kind, j, dst in acts:
                if kind == "act":
                    nc.scalar.activation(dst, Xv, TANH,
                                         bias=ms[:, j - 1:j], scale=T_SCALE)
                else:
                    nc.vector.tensor_scalar(
                        dst, Xv, ms[:, NT + (j - NT - 1):NT + (j - NT)],
                        0.0, ADD, MAX)

            # ---- reduce + C-mix (after remote data arrives).  G8 is split
            # into its two contiguous c-group halves (cores 0-3, 4-7); the
            # d-half selection happens in the h-masked mix stationaries.
            tps = psp.tile([88, 2 * 128], F32, tag="tps")
            tr0 = None
            for grp in range(2):
                tr = nc.tensor.transpose(tps[:, grp * 128:(grp + 1) * 128],
                                         G8[:, grp * 88:(grp + 1) * 88],
                                         ident[:])
                bass._add_dep_helper(tr.ins, trig.ins, sync=True,
                                     reason="transpose after remote trigger")
                if tr0 is None:
                    tr0 = tr
                nc.vector.tensor_copy(tg8[0:88, grp * 128:(grp + 1) * 128],
                                      tps[:, grp * 128:(grp + 1) * 128])
            hps = psp.tile([12, 2 * 128], F32, tag="hps")
            for h in range(2):
                for grp in range(2):
                    blk = (h * 2 + grp) * 12
                    nc.tensor.matmul(hps[:, h * 128:(h + 1) * 128],
                                     cts[:, blk:blk + 12],
                                     tg8[:, grp * 128:(grp + 1) * 128],
                                     start=(grp == 0), stop=(grp == 1))
            hsb = cst.tile([12, 2 * 128], F32)
            nc.vector.tensor_copy(hsb[:], hps[:])
            c0m = cst.tile([1, D], F32)
            nc.vector.tensor_tensor(c0m[:], hps[0:1, :], wrow[:], MULT)
            c0d = cst.tile([1, 1], F32)
            nc.vector.tensor_reduce(c0d[:], c0m[:],
                                    axis=mybir.AxisListType.XYZW, op=ADD)
            c0t = cst.tile([1, 1], F32)
            nc.vector.tensor_tensor(c0t[:], c0d[:], ms[0:1, 14:15], ADD)

            hTp = psp.tile([128, 2 * 12], F32, tag="hTp")
            for h in range(2):
                nc.tensor.transpose(hTp[:, h * 12:(h + 1) * 12],
                                    hsb[:, h * 128:(h + 1) * 128],
                                    ident[0:12, 0:12])
            # pack + w-scale: hTp cols = j' 0..11. Odd j' (j=1,3,5,7,9 + solo
            # j=11) land at packed cols 0..5; even j' (const j'=0 at the
            # harmless col 31, then j=2,4,6,8,10) at cols 31..36.
            for h in range(2):
                hv = hTp[:, h * 12:(h + 1) * 12].rearrange(
                    "p (jp par) -> p par jp", par=2)
                wcol = ms[:, 12 + h:13 + h]
                nc.vector.tensor_scalar(hT[:, h * PACKW:h * PACKW + 6],
                                        hv[:, 1, :], wcol, None, MULT)
                nc.vector.tensor_scalar(hT[:, h * PACKW + 31:h * PACKW + 37],
                                        hv[:, 0, :], wcol, None, MULT)

            # ---- contraction: ps2[0, 0:256] + ps2[32, 256:512] are wanted
            ps2 = psp.tile([33, 2 * NA], F32, tag="ps2")
            mm = []
            for q in [0, 4]:
                for h in range(2):
                    mm.append((hT[:, h * PACKW + q:h * PACKW + q + 33],
                               fvp[q][:, h * 512:(h + 1) * 512], slice(0, 512)))
            for h in range(2):
                mm.append((hT[:, h * PACKW + 5:h * PACKW + 5 + 33],
                           fvs[:, h * NA:(h + 1) * NA], slice(0, 256)))
            for q in [1, 2, 3]:
                for h in range(2):
                    mm.append((hT[:, h * PACKW + q:h * PACKW + q + 33],
                               fvp[q][:, h * 512:(h + 1) * 512], slice(0, 512)))
            for i, (st, mv, osl) in enumerate(mm):
                nc.tensor.matmul(ps2[:, osl], st, mv,
                                 start=(i == 0), stop=(i == len(mm) - 1))

            oeven = cst.tile([1, NA], F32)
            nc.vector.tensor_copy(oeven[:], ps2[32:33, NA:2 * NA])
            orow = cst.tile([1, NA], F32)
            nc.vector.scalar_tensor_tensor(
                orow[:], ps2[0:1, 0:NA], c0t[:], oeven[:], ADD, ADD)
            nc.sync.dma_start(out_d[:], orow[:])

    # ---- post-scheduling: splice in the cross-core waits the tile sim
    # cannot model (kernel-entry barrier before the send trigger; remote-sem
    # wait before the first reader of G8).
    def insert_wait_before(consumer, wait_inst):
        src = dst = None
        for blk in nc.main_func.blocks:
            if wait_inst.ins in blk.instructions:
                src = blk
            if consumer.ins in blk.instructions:
                dst = blk
        assert src is not None and dst is not None
        src.instructions.remove(wait_inst.ins)
        dst.instructions.insert(dst.instructions.index(consumer.ins),
                                wait_inst.ins)

    wbar = nc.gpsimd.bir_kernel_barrier_wait([list(range(8))])
    insert_wait_before(trig, wbar)
    wrs = nc.tensor.wait_ge(rs, 16)
    insert_wait_before(tr0, wrs)
    nc.finalize()
    return nc


_CACHE = {}
LAST_EXEC_NS = None


def kernel(**inputs):
    global LAST_EXEC_NS
    A = np.asarray(inputs["actor_embeddings"], np.float32)
    B = np.asarray(inputs["bill_embeddings"], np.float32)
    outc = np.asarray(inputs["bill_outcomes"], np.float32)
    Wa = np.asarray(inputs["W_actor"], np.float32)
    ba = np.asarray(inputs["b_actor"], np.float32)
    Wb = np.asarray(inputs["W_bill"], np.float32)
    bb = np.asarray(inputs["b_bill"], np.float32)
    w2 = np.asarray(inputs["w_score"], np.float32)
    b_score = float(np.asarray(inputs["b_score"], np.float32))

    t_sh, r_off = _basis_params()
    C = _coeffs()

    if "nc" not in _CACHE:
        _CACHE["nc"] = _build()
    nc = _CACHE["nc"]

    wa_p = _bf16(_pack_ktiles(Wa))
    wb_p = _bf16(_pack_ktiles(Wb))
    rvb = np.concatenate([ba, bb]).reshape(1, 2 * D)
    rvb = np.ascontiguousarray(_bf16(rvb))

    # cts: 4 mix stationaries, one per (h, c-group).  tg8 rows carry
    # (c_local, hh, jj); the stationary keeps only hh == h rows.  Row 96 (the
    # tg8 ones row, group a only) folds the constant bill feature
    # g_0 = sum(outc); col = j' 0..11 (col 0 -> c0).  1/M folded here.
    cts = np.zeros((128, 48), np.float64)
    for h in range(2):
        for grp in range(2):
            blk = (h * 2 + grp) * 12
            for cl in range(4):
                for jj in range(11):
                    cts[cl * 22 + h * 11 + jj, blk:blk + 12] = C[:, jj + 1] / M
            if grp == 0:
                cts[96, blk:blk + 12] = C[:, 0] * float(outc.sum()) / M
    cts = np.ascontiguousarray(_bf16(cts))

    ms = np.zeros((128, MSW), np.float32)
    ms[:, 0:NT] = t_sh[None, :]
    ms[:, NT:NT + NR] = r_off[None, :]
    ms[:, 12] = w2[0:128]
    ms[:, 13] = w2[128:256]
    ms[0, 14] = b_score * float(outc.mean())

    in_maps = []
    for c in range(N_CORES):
        msc = ms.copy()
        msc[:, 11] = outc[c * NB:(c + 1) * NB]
        in_maps.append({
            "BT": _bf16(_pack_ktiles(B[c * NB:(c + 1) * NB].T.copy())),
            "Wb": wb_p,
            "AT": _bf16(_pack_ktiles(A[c * NA:(c + 1) * NA].T.copy())),
            "Wa": wa_p,
            "misc": np.ascontiguousarray(msc),
            "rvb": rvb,
            "wrow": np.ascontiguousarray(w2.reshape(1, D)),
            "cts": cts,
        })
    trace = bool(os.environ.get("KERNEL_TRACE"))
    r = run_bass_kernel_spmd(nc, in_maps, list(range(N_CORES)), trace=trace)
    out = np.concatenate([res["out"].reshape(NA) for res in r.results])
    if trace:
        LAST_EXEC_NS = (r.exec_time_ns, 0)
    return out.astype(np.float32)
